# revision 59
# baseline (speedup 1.0000x reference)
"""Single-head causal attention on 8 TRN2 NeuronCores.

Problem: x[8,2048,1024] @ Wq/Wk/Wv[1024,64] -> causal softmax attention -> out[8,2048,64].
Sharding: data-parallel over batch B=8, one batch element per core; weights replicated.

Per-core design (T=2048, C=1024, H=64), tuned for dense PE occupancy
(253us staged baseline -> ~69us):
 - x cast to bf16 on the host (halving the HBM read; the kernel casts to
   bf16 anyway so numerics are identical) and loaded per 128-row block on the
   sync HWDGE ring in consumption order (weights host-packed so their DMA is
   contiguous and cannot stall the ring), then transposed on PE into xT tiles.
 - PE warmup: dummy transposes on a memset tile flip the HAM clock gate to
   2.4GHz before real work arrives (otherwise the whole kernel runs at 1.2GHz).
 - transposes run in two tt-major passes of two jc-pairs so each arriving
   block unlocks 4 consecutive PE-FIFO transposes (no head-of-line blocking);
   each [128,1024]bf16 staging bank drained by one wide DVE copy.
 - chunk stages software-pipelined: chunk tb+1's transposes are emitted
   between chunk tb's projection and score phases so they fill the
   projection-copy WAR stalls. Engine FIFOs are strict, so program order of
   same-engine ops is chosen to match true dependency order (casts interleaved
   between xt copies; weight casts after chunk-0 casts).
 - q,k projected together (stationary [Wq|Wk]) into per-chunk qT/kT tiles
   (no cross-chunk WAR).
 - scores TRANSPOSED: weiT[s,t] = kT.T@qT per (s-block, t-chunk); two s-blocks
   per [128,1024] f32 PSUM tile so each ScalarE exp covers up to 1024 cols
   (halves ACT instruction overhead). exp folds in the C**-0.5 scale; no max
   subtraction (scores O(1), softmax shift-invariant).
 - causal mask: fully-masked blocks skipped, score matmuls and PV stream only
   [lo:] of diagonal tiles, below-diagonal of the 128x128 diagonal zeroed by
   GpSimd affine_select; softmax denominator via an extra ones column on the
   PV stationary [v | 1].
 - final normalization (divide by sums + transpose [65,512]) on host.
"""

import numpy as np

import concourse.bass as bass
import concourse.mybir as mybir
import concourse.tile as tile
from concourse import bacc
from concourse.masks import make_identity
from contextlib import ExitStack

P = 128
T = 2048
C = 1024
H = 64
B = 8
NC = C // P          # 8 c-tiles
NT = T // P          # 16 s/t 128-blocks
CH = 512             # t-chunk width
NCH = T // CH        # 4 chunks
BPC = CH // P        # 4 blocks per chunk
SCALE = float(C) ** -0.5
F32 = mybir.dt.float32
BF16 = mybir.dt.bfloat16
EXP = mybir.ActivationFunctionType.Exp
N_WARM = 10          # PE warmup transposes (bf16 x arrives ~8.5us)


def build_nc():
    nc = bacc.Bacc(None, target_bir_lowering=False)
    # x pre-cast to bf16 on host: halves the HBM read (the kernel would cast
    # to bf16 on-chip anyway, so numerics are identical)
    x = nc.dram_tensor("xb16", [T, C], BF16, kind="ExternalInput")
    # weights pre-packed on host into the on-chip layout (contiguous DMA)
    wqk_d = nc.dram_tensor("Wqk", [P, NC, 2 * H], F32, kind="ExternalInput")
    wv_d = nc.dram_tensor("Wv2", [P, NC, H], F32, kind="ExternalInput")
    out_d = nc.dram_tensor("outT", [H + 1, T], F32, kind="ExternalOutput")

    with tile.TileContext(nc) as tc, ExitStack() as ctx:
        consts = ctx.enter_context(tc.tile_pool(name="consts", bufs=1))
        xbp = ctx.enter_context(tc.tile_pool(name="xbp", bufs=16))
        xcp = ctx.enter_context(tc.tile_pool(name="xcp", bufs=16))
        xtp = ctx.enter_context(tc.tile_pool(name="xtp", bufs=8))
        persist = ctx.enter_context(tc.tile_pool(name="persist", bufs=1))
        wei = ctx.enter_context(tc.tile_pool(name="wei", bufs=8))
        vtsp = ctx.enter_context(tc.tile_pool(name="vtsp", bufs=2))
        fin = ctx.enter_context(tc.tile_pool(name="fin", bufs=2))
        # PSUM: 8 banks total; ptx 2 + ppj 1 + psc 2x2 + pout 1 = 8.
        ptx = ctx.enter_context(tc.tile_pool(name="ptx", bufs=2, space="PSUM"))
        ppj = ctx.enter_context(tc.tile_pool(name="ppj", bufs=1, space="PSUM"))
        psc = ctx.enter_context(tc.tile_pool(name="psc", bufs=2, space="PSUM"))
        pout = ctx.enter_context(tc.tile_pool(name="pout", bufs=1, space="PSUM"))

        # ---- PE warmup first: a single-memset source so the dummy transposes
        # (which flip the HAM clock gate to 2.4GHz) start as early as possible
        warm_b = consts.tile([P, P], BF16)
        nc.gpsimd.memset(warm_b, 1.0)
        for _ in range(N_WARM):
            wt = ptx.tile([P, 2 * CH], BF16, tag="tr")
            nc.tensor.transpose(wt[:, 0:P], warm_b, warm_b)

        # ---- x block loads on the sync HWDGE ring (in consumption order).
        # The first 8 blocks are split into column halves (aligned with the
        # two transpose passes) to halve delivery granularity at the front.
        NSW = 0   # (cast-DMA offload tried and regressed; keep all on sync)
                  # queue: the sync ring finishes blocks 0-7 sooner (they pace
                  # the early chunks) and DVE sheds half the casts
        xbh = [None] * NT
        xc = [None] * NT

        def load_block(blk):
            rows = x[blk * P : (blk + 1) * P, :]
            c_ = xcp.tile([P, C], BF16, tag="xc", name=f"xc{blk}", bufs=NT)
            nc.sync.dma_start(out=c_, in_=rows)
            xc[blk] = c_

        for blk in range(BPC):
            load_block(blk)

        # ---- constants
        ident_f = consts.tile([P, P], F32)
        make_identity(nc, ident_f)
        ident_b = consts.tile([P, P], BF16)
        nc.vector.tensor_copy(out=ident_b, in_=ident_f)

        # weights: host-packed layout, two small contiguous HWDGE loads
        wqk_f = consts.tile([P, NC, 2 * H], F32)
        wv_f = consts.tile([P, NC, H], F32)
        nc.sync.dma_start(out=wqk_f, in_=wqk_d[:, :, :])
        nc.sync.dma_start(out=wv_f, in_=wv_d[:, :, :])

        # remaining x blocks
        for blk in range(BPC, NT):
            load_block(blk)

        # ---- f32 -> bf16 casts on DVE (2x mode) for the sync-ring blocks;
        # chunk 0 upfront, the rest issued just-in-time inside the previous
        # chunk's body so urgent DVE copies are not stuck behind them
        NF32 = 0  # (f32-direct transposes tried and regressed; keep casts)
                  # on delivery there anyway and the 2cyc/row stream hides
                  # under LDWEIGHTS); later blocks get bf16 casts since their
                  # transposes overlap busy PE phases

        def cast_block(blk):
            return  # x arrives bf16; nothing to cast

        def xsrc(blk, jc):
            """(source slice, identity) for the transpose of (blk, c-tile jc)"""
            return xc[blk][:, jc * P : (jc + 1) * P], ident_b

        # weight casts after the chunk-0 x casts (DVE FIFO order matters)
        wqk_sb = consts.tile([P, NC, P], BF16)
        nc.vector.tensor_copy(out=wqk_sb, in_=wqk_f)
        wv_sb = consts.tile([P, NC, H], BF16)
        nc.vector.tensor_copy(out=wv_sb, in_=wv_f)

        # per-chunk persistent projections (separate tiles -> no cross-chunk WAR)
        qT_c = [persist.tile([H, CH], BF16, tag=f"qT{tb}", name=f"qT{tb}") for tb in range(NCH)]
        kT_c = [persist.tile([H, CH], BF16, tag=f"kT{tb}", name=f"kT{tb}") for tb in range(NCH)]
        v_c = [persist.tile([P, BPC, H + 1], BF16, tag=f"v{tb}", name=f"v{tb}") for tb in range(NCH)]
        for tb in range(NCH):
            nc.gpsimd.memset(v_c[tb][:, :, H : H + 1], 1.0)  # denominator column

        xt_all = [None] * NCH

        def stage_A(tb):
            """transposes + xt copies (+ next-chunk casts interleaved).
            Early chunks (f32-resident blocks) transpose straight from f32
            into f32 PSUM tiles, one jc per bank, with the f32->bf16 cast
            folded into the PSUM->SBUF copy; later chunks transpose bf16
            jc-pairs per bank."""
            blk0 = tb * BPC
            xt8 = [None] * NC
            if tb == 0:
                # chunk 0: borrow the (still idle) score PSUM banks so all four
                # staging tiles are live -> strict tt-major, 8 transposes
                # unlocked per arriving block, no head-of-line blocking
                pts = [ptx.tile([P, 2 * CH], BF16, tag="tr", name="pt0_0"),
                       ptx.tile([P, 2 * CH], BF16, tag="tr", name="pt0_1"),
                       psc.tile([P, 2 * CH], BF16, tag="sc", name="pt0_2"),
                       psc.tile([P, 2 * CH], BF16, tag="sc", name="pt0_3")]
                for tt in range(BPC):
                    for jp in range(NC // 2):
                        for h in range(2):
                            jc = 2 * jp + h
                            src, idn = xsrc(blk0 + tt, jc)
                            nc.tensor.transpose(
                                pts[jp][:, h * CH + tt * P : h * CH + (tt + 1) * P],
                                src, idn,
                            )
                for jp in range(NC // 2):
                    t_ = xtp.tile([P, 2 * CH], BF16, tag="xt", name=f"xt{jp}")
                    nc.vector.tensor_copy(out=t_, in_=pts[jp])
                    xt8[2 * jp] = t_[:, 0:CH]
                    xt8[2 * jp + 1] = t_[:, CH : 2 * CH]
                xt_all[tb] = xt8
                return
            if blk0 < NF32:
                for pr in range(NC // 2):
                    pts = [ptx.tile([P, CH], F32, tag="tr", name=f"ptf{tb}_{j}")
                           for j in (2 * pr, 2 * pr + 1)]
                    for tt in range(BPC):
                        for pi, jc in enumerate((2 * pr, 2 * pr + 1)):
                            nc.tensor.transpose(
                                pts[pi][:, tt * P : (tt + 1) * P],
                                xbh[blk0 + tt][:, jc * P : (jc + 1) * P],
                                ident_f,
                            )
                    for pi, jc in enumerate((2 * pr, 2 * pr + 1)):
                        t_ = xtp.tile([P, CH], BF16, tag="xt1", name=f"xt1_{jc}",
                                      bufs=2 * NC)
                        nc.vector.tensor_copy(out=t_, in_=pts[pi])  # casts
                        xt8[jc] = t_
                    if tb + 1 < NCH:
                        cast_block((tb + 1) * BPC + pr)
            else:
                for half in range(2):
                    pts = [ptx.tile([P, 2 * CH], BF16, tag="tr", name=f"pt{tb}_{jp}")
                           for jp in (2 * half, 2 * half + 1)]
                    for tt in range(BPC):
                        for pi, jp in enumerate((2 * half, 2 * half + 1)):
                            for h in range(2):
                                jc = 2 * jp + h
                                src, idn = xsrc(blk0 + tt, jc)
                                nc.tensor.transpose(
                                    pts[pi][:, h * CH + tt * P : h * CH + (tt + 1) * P],
                                    src, idn,
                                )
                    for pi, jp in enumerate((2 * half, 2 * half + 1)):
                        t_ = xtp.tile([P, 2 * CH], BF16, tag="xt", name=f"xt{jp}")
                        nc.vector.tensor_copy(out=t_, in_=pts[pi])
                        xt8[2 * jp] = t_[:, 0:CH]
                        xt8[2 * jp + 1] = t_[:, CH : 2 * CH]
                        if tb + 1 < NCH:
                            cast_block((tb + 1) * BPC + 2 * half + pi)
            xt_all[tb] = xt8

        vts_all = [None] * NCH

        def stage_B(tb):
            """q|k and v projections + PSUM->SBUF copies"""
            xt = xt_all[tb]
            pj_copy = nc.scalar.copy if tb < 2 else nc.vector.tensor_copy
            pqk = ppj.tile([P, CH], F32, tag="pj")
            for jc in range(NC):
                nc.tensor.matmul(pqk, lhsT=wqk_sb[:, jc, :], rhs=xt[jc],
                                 start=(jc == 0), stop=(jc == NC - 1))
            pj_copy(out=qT_c[tb], in_=pqk[0:H, :])
            pj_copy(out=kT_c[tb], in_=pqk[H : 2 * H, :])
            pv = ppj.tile([P, CH], F32, tag="pj")
            for jc in range(NC):
                nc.tensor.matmul(pv[0:H, :], lhsT=wv_sb[:, jc, :], rhs=xt[jc],
                                 start=(jc == 0), stop=(jc == NC - 1))
            vts = vtsp.tile([H, CH], BF16, tag="vt")
            pj_copy(out=vts, in_=pv[0:H, :])
            vts_all[tb] = vts

        def stage_vt(tb):
            """small transposes to v natural [s, 64] (vts copy has settled)"""
            vts = vts_all[tb]
            pvn = ptx.tile([P, 2 * CH], BF16, tag="tr")
            for tt in range(BPC):
                nc.tensor.transpose(pvn[:, tt * H : (tt + 1) * H],
                                    vts[:, tt * P : (tt + 1) * P],
                                    ident_b[0:H, 0:H])
            nc.vector.tensor_copy(out=v_c[tb][:, :, 0:H], in_=pvn[:, 0 : BPC * H])

        def stage_C(tb):
            """scores (2 s-blocks per [128,1024] f32 tile) + exp + mask + PV;
            diagonal s-blocks first so the chunk's closing exp->PV chain has no
            mask step and the irregular work happens early"""
            po = pout.tile([H + 1, CH], F32, tag="po")
            nsb = (tb + 1) * BPC
            sis = list(range(nsb))
            for g in range(nsb // 2):
                pair = sis[2 * g : 2 * g + 2]
                ps = psc.tile([P, 2 * CH], F32, tag="sc")
                w = wei.tile([P, 2 * CH], BF16, tag="w")
                los = []
                for m, si in enumerate(pair):
                    lo = max(0, (si - tb * BPC) * P)
                    los.append(lo)
                    nc.tensor.matmul(
                        ps[:, m * CH + lo : (m + 1) * CH],
                        lhsT=kT_c[si // BPC][:, (si % BPC) * P : (si % BPC + 1) * P],
                        rhs=qT_c[tb][:, lo:CH],
                        start=True, stop=True,
                    )
                # one exp over both blocks (the sub-lo slice of block m=1 is
                # garbage but never read; PV streams only [lo:]) — except the
                # kernel's very last pair, split to shorten the exp->PV tail
                last_pair = (tb == NCH - 1) and (g == nsb // 2 - 1)
                if not last_pair:
                    nc.scalar.activation(out=w[:, los[0] : 2 * CH],
                                         in_=ps[:, los[0] : 2 * CH],
                                         func=EXP, scale=SCALE)
                for m, si in enumerate(pair):
                    lo = los[m]
                    if last_pair:
                        nc.scalar.activation(out=w[:, m * CH + lo : (m + 1) * CH],
                                             in_=ps[:, m * CH + lo : (m + 1) * CH],
                                             func=EXP, scale=SCALE)
                    if si >= tb * BPC:  # diagonal block: zero below-diagonal (t < s)
                        nc.gpsimd.affine_select(
                            out=w[:, m * CH + lo : m * CH + lo + P],
                            in_=w[:, m * CH + lo : m * CH + lo + P],
                            compare_op=mybir.AluOpType.is_ge,
                            fill=0.0,
                            base=0,
                            # keep where (col - row) >= 0
                            pattern=[[1, P]],
                            channel_multiplier=-1,
                        )
                    nc.tensor.matmul(po[:, lo:CH], lhsT=v_c[si // BPC][:, si % BPC, :],
                                     rhs=w[:, m * CH + lo : (m + 1) * CH],
                                     start=(g == 0 and m == 0), stop=(g == nsb // 2 - 1 and m == 1))
            os_ = fin.tile([H + 1, CH], F32, tag="ot")
            if tb == NCH - 1:
                nc.scalar.copy(out=os_, in_=po)
            else:
                nc.vector.tensor_copy(out=os_, in_=po)
            nc.sync.dma_start(out=out_d[:, tb * CH : (tb + 1) * CH], in_=os_)

        # software-pipelined emission: chunk tb+1's transposes sit between
        # chunk tb's projection and score phases in the PE FIFO, covering the
        # projection-copy WAR stalls
        stage_A(0)
        stage_A(1)
        stage_B(0)
        for tb in range(NCH):
            if tb + 2 < NCH:
                stage_A(tb + 2)
            stage_vt(tb)
            stage_C(tb)
            if tb + 1 < NCH:
                stage_B(tb + 1)
    return nc


_NC_CACHE = []


def _get_nc():
    if not _NC_CACHE:
        nc = build_nc()
        nc.finalize()  # bacc compile: register allocation, DCE
        _NC_CACHE.append(nc)
    return _NC_CACHE[0]


def make_in_maps(inputs):
    import ml_dtypes
    x = np.ascontiguousarray(
        np.asarray(inputs["x"], dtype=np.float32).astype(ml_dtypes.bfloat16))
    wq = np.asarray(inputs["Wq"], dtype=np.float32)
    wk = np.asarray(inputs["Wk"], dtype=np.float32)
    wv = np.asarray(inputs["Wv"], dtype=np.float32)
    # host-side repack into the on-chip stationary layout [p, jc, h]
    wqk = np.ascontiguousarray(np.concatenate(
        [wq.reshape(NC, P, H).transpose(1, 0, 2), wk.reshape(NC, P, H).transpose(1, 0, 2)],
        axis=2))
    wv2 = np.ascontiguousarray(wv.reshape(NC, P, H).transpose(1, 0, 2))
    return [{"xb16": np.ascontiguousarray(x[b]), "Wqk": wqk, "Wv2": wv2} for b in range(B)]


def kernel(**inputs):
    from concourse.bass_utils import run_bass_kernel_spmd

    nc = _get_nc()
    res = run_bass_kernel_spmd(nc, make_in_maps(inputs), core_ids=list(range(B)))
    return postprocess([res.results[b]["outT"] for b in range(B)])


def postprocess(outTs):
    outs = []
    for oT in outTs:
        outs.append((oT[0:H, :] / oT[H : H + 1, :]).T.astype(np.float32))
    return np.stack(outs, axis=0)


if __name__ == "__main__":
    import os
    os.makedirs("/tmp/neffdir3", exist_ok=True)
    from concourse.bass_utils import compile_bass_kernel

    nc = _get_nc()
    print("build OK, instructions:",
          sum(len(bb.instructions) for bb in nc.m.functions[0].blocks))
    print("COMPILED:", compile_bass_kernel(nc, "/tmp/neffdir3"))


# revision 60
# speedup vs baseline: 1.0515x; 1.0515x over previous
"""Single-head causal attention on 8 TRN2 NeuronCores.

Problem: x[8,2048,1024] @ Wq/Wk/Wv[1024,64] -> causal softmax attention -> out[8,2048,64].
Sharding: data-parallel over batch B=8, one batch element per core; weights replicated.

Per-core design (T=2048, C=1024, H=64), tuned for dense PE occupancy
(253us staged baseline -> ~69us):
 - x cast to bf16 on the host (halving the HBM read; the kernel casts to
   bf16 anyway so numerics are identical) and loaded per 128-row block on the
   sync HWDGE ring in consumption order (weights host-packed so their DMA is
   contiguous and cannot stall the ring), then transposed on PE into xT tiles.
 - PE warmup: dummy transposes on a memset tile flip the HAM clock gate to
   2.4GHz before real work arrives (otherwise the whole kernel runs at 1.2GHz).
 - transposes run in two tt-major passes of two jc-pairs so each arriving
   block unlocks 4 consecutive PE-FIFO transposes (no head-of-line blocking);
   each [128,1024]bf16 staging bank drained by one wide DVE copy.
 - chunk stages software-pipelined: chunk tb+1's transposes are emitted
   between chunk tb's projection and score phases so they fill the
   projection-copy WAR stalls. Engine FIFOs are strict, so program order of
   same-engine ops is chosen to match true dependency order (casts interleaved
   between xt copies; weight casts after chunk-0 casts).
 - q,k projected together (stationary [Wq|Wk]) into per-chunk qT/kT tiles
   (no cross-chunk WAR).
 - scores TRANSPOSED: weiT[s,t] = kT.T@qT per (s-block, t-chunk); two s-blocks
   per [128,1024] f32 PSUM tile so each ScalarE exp covers up to 1024 cols
   (halves ACT instruction overhead). exp folds in the C**-0.5 scale; no max
   subtraction (scores O(1), softmax shift-invariant).
 - causal mask: fully-masked blocks skipped, score matmuls and PV stream only
   [lo:] of diagonal tiles, below-diagonal of the 128x128 diagonal zeroed by
   GpSimd affine_select; softmax denominator via an extra ones column on the
   PV stationary [v | 1].
 - final normalization (divide by sums + transpose [65,512]) on host.
"""

import numpy as np

import concourse.bass as bass
import concourse.mybir as mybir
import concourse.tile as tile
from concourse import bacc
from concourse.masks import make_identity
from contextlib import ExitStack

P = 128
T = 2048
C = 1024
H = 64
B = 8
NC = C // P          # 8 c-tiles
NT = T // P          # 16 s/t 128-blocks
CH = 512             # t-chunk width
NCH = T // CH        # 4 chunks
BPC = CH // P        # 4 blocks per chunk
SCALE = float(C) ** -0.5
F32 = mybir.dt.float32
BF16 = mybir.dt.bfloat16
EXP = mybir.ActivationFunctionType.Exp
N_WARM = 8           # PE warmup transposes (bf16 x arrives ~8.5us)


def build_nc():
    nc = bacc.Bacc(None, target_bir_lowering=False)
    # x pre-cast to bf16 on host: halves the HBM read (the kernel would cast
    # to bf16 on-chip anyway, so numerics are identical)
    x = nc.dram_tensor("xb16", [T, C], BF16, kind="ExternalInput")
    # weights pre-packed on host into the on-chip layout (contiguous DMA)
    wqk_d = nc.dram_tensor("Wqk", [P, NC, 2 * H], F32, kind="ExternalInput")
    wv_d = nc.dram_tensor("Wv2", [P, NC, H], F32, kind="ExternalInput")
    out_d = nc.dram_tensor("outT", [H + 1, T], F32, kind="ExternalOutput")

    with tile.TileContext(nc) as tc, ExitStack() as ctx:
        consts = ctx.enter_context(tc.tile_pool(name="consts", bufs=1))
        xbp = ctx.enter_context(tc.tile_pool(name="xbp", bufs=16))
        xcp = ctx.enter_context(tc.tile_pool(name="xcp", bufs=16))
        xtp = ctx.enter_context(tc.tile_pool(name="xtp", bufs=8))
        persist = ctx.enter_context(tc.tile_pool(name="persist", bufs=1))
        wei = ctx.enter_context(tc.tile_pool(name="wei", bufs=8))
        vtsp = ctx.enter_context(tc.tile_pool(name="vtsp", bufs=2))
        fin = ctx.enter_context(tc.tile_pool(name="fin", bufs=2))
        # PSUM: 8 banks total; ptx 2 + ppj 1 + psc 2x2 + pout 1 = 8.
        ptx = ctx.enter_context(tc.tile_pool(name="ptx", bufs=2, space="PSUM"))
        ppj = ctx.enter_context(tc.tile_pool(name="ppj", bufs=1, space="PSUM"))
        psc = ctx.enter_context(tc.tile_pool(name="psc", bufs=2, space="PSUM"))
        pout = ctx.enter_context(tc.tile_pool(name="pout", bufs=1, space="PSUM"))

        # ---- PE warmup first: a single-memset source so the dummy transposes
        # (which flip the HAM clock gate to 2.4GHz) start as early as possible
        warm_b = consts.tile([P, P], BF16)
        nc.gpsimd.memset(warm_b, 1.0)
        for _ in range(N_WARM):
            wt = ptx.tile([P, 2 * CH], BF16, tag="tr")
            nc.tensor.transpose(wt[:, 0:P], warm_b, warm_b)

        # ---- x block loads on the sync HWDGE ring (in consumption order).
        # The first 8 blocks are split into column halves (aligned with the
        # two transpose passes) to halve delivery granularity at the front.
        NSW = 0   # (cast-DMA offload tried and regressed; keep all on sync)
                  # queue: the sync ring finishes blocks 0-7 sooner (they pace
                  # the early chunks) and DVE sheds half the casts
        xbh = [None] * NT
        xc = [None] * NT

        def load_block(blk):
            rows = x[blk * P : (blk + 1) * P, :]
            c_ = xcp.tile([P, C], BF16, tag="xc", name=f"xc{blk}", bufs=NT)
            nc.sync.dma_start(out=c_, in_=rows)
            xc[blk] = c_

        for blk in range(BPC):
            load_block(blk)

        # ---- constants
        ident_f = consts.tile([P, P], F32)
        make_identity(nc, ident_f)
        ident_b = consts.tile([P, P], BF16)
        nc.vector.tensor_copy(out=ident_b, in_=ident_f)

        # weights: host-packed layout, two small contiguous HWDGE loads
        wqk_f = consts.tile([P, NC, 2 * H], F32)
        wv_f = consts.tile([P, NC, H], F32)
        nc.sync.dma_start(out=wqk_f, in_=wqk_d[:, :, :])
        nc.sync.dma_start(out=wv_f, in_=wv_d[:, :, :])

        # remaining x blocks
        for blk in range(BPC, NT):
            load_block(blk)

        # ---- f32 -> bf16 casts on DVE (2x mode) for the sync-ring blocks;
        # chunk 0 upfront, the rest issued just-in-time inside the previous
        # chunk's body so urgent DVE copies are not stuck behind them
        NF32 = 0  # (f32-direct transposes tried and regressed; keep casts)
                  # on delivery there anyway and the 2cyc/row stream hides
                  # under LDWEIGHTS); later blocks get bf16 casts since their
                  # transposes overlap busy PE phases

        def cast_block(blk):
            return  # x arrives bf16; nothing to cast

        def xsrc(blk, jc):
            """(source slice, identity) for the transpose of (blk, c-tile jc)"""
            return xc[blk][:, jc * P : (jc + 1) * P], ident_b

        # weight casts after the chunk-0 x casts (DVE FIFO order matters)
        wqk_sb = consts.tile([P, NC, P], BF16)
        nc.vector.tensor_copy(out=wqk_sb, in_=wqk_f)
        wv_sb = consts.tile([P, NC, H], BF16)
        nc.vector.tensor_copy(out=wv_sb, in_=wv_f)

        # per-chunk persistent projections (separate tiles -> no cross-chunk WAR)
        qT_c = [persist.tile([H, CH], BF16, tag=f"qT{tb}", name=f"qT{tb}") for tb in range(NCH)]
        kT_c = [persist.tile([H, CH], BF16, tag=f"kT{tb}", name=f"kT{tb}") for tb in range(NCH)]
        v_c = [persist.tile([P, BPC, H + 1], BF16, tag=f"v{tb}", name=f"v{tb}") for tb in range(NCH)]
        for tb in range(NCH):
            nc.gpsimd.memset(v_c[tb][:, :, H : H + 1], 1.0)  # denominator column

        xt_all = [None] * NCH

        def stage_A(tb):
            """transposes + xt copies (+ next-chunk casts interleaved).
            Early chunks (f32-resident blocks) transpose straight from f32
            into f32 PSUM tiles, one jc per bank, with the f32->bf16 cast
            folded into the PSUM->SBUF copy; later chunks transpose bf16
            jc-pairs per bank."""
            blk0 = tb * BPC
            xt8 = [None] * NC
            if tb == 0:
                # chunk 0: borrow the (still idle) score PSUM banks so all four
                # staging tiles are live -> strict tt-major, 8 transposes
                # unlocked per arriving block, no head-of-line blocking
                pts = [ptx.tile([P, 2 * CH], BF16, tag="tr", name="pt0_0"),
                       ptx.tile([P, 2 * CH], BF16, tag="tr", name="pt0_1"),
                       psc.tile([P, 2 * CH], BF16, tag="sc", name="pt0_2"),
                       psc.tile([P, 2 * CH], BF16, tag="sc", name="pt0_3")]
                for tt in range(BPC):
                    for jp in range(NC // 2):
                        for h in range(2):
                            jc = 2 * jp + h
                            src, idn = xsrc(blk0 + tt, jc)
                            nc.tensor.transpose(
                                pts[jp][:, h * CH + tt * P : h * CH + (tt + 1) * P],
                                src, idn,
                            )
                for jp in range(NC // 2):
                    t_ = xtp.tile([P, 2 * CH], BF16, tag="xt", name=f"xt{jp}")
                    nc.vector.tensor_copy(out=t_, in_=pts[jp])
                    xt8[2 * jp] = t_[:, 0:CH]
                    xt8[2 * jp + 1] = t_[:, CH : 2 * CH]
                xt_all[tb] = xt8
                return
            if blk0 < NF32:
                for pr in range(NC // 2):
                    pts = [ptx.tile([P, CH], F32, tag="tr", name=f"ptf{tb}_{j}")
                           for j in (2 * pr, 2 * pr + 1)]
                    for tt in range(BPC):
                        for pi, jc in enumerate((2 * pr, 2 * pr + 1)):
                            nc.tensor.transpose(
                                pts[pi][:, tt * P : (tt + 1) * P],
                                xbh[blk0 + tt][:, jc * P : (jc + 1) * P],
                                ident_f,
                            )
                    for pi, jc in enumerate((2 * pr, 2 * pr + 1)):
                        t_ = xtp.tile([P, CH], BF16, tag="xt1", name=f"xt1_{jc}",
                                      bufs=2 * NC)
                        nc.vector.tensor_copy(out=t_, in_=pts[pi])  # casts
                        xt8[jc] = t_
                    if tb + 1 < NCH:
                        cast_block((tb + 1) * BPC + pr)
            else:
                for half in range(2):
                    pts = [ptx.tile([P, 2 * CH], BF16, tag="tr", name=f"pt{tb}_{jp}")
                           for jp in (2 * half, 2 * half + 1)]
                    for tt in range(BPC):
                        for pi, jp in enumerate((2 * half, 2 * half + 1)):
                            for h in range(2):
                                jc = 2 * jp + h
                                src, idn = xsrc(blk0 + tt, jc)
                                nc.tensor.transpose(
                                    pts[pi][:, h * CH + tt * P : h * CH + (tt + 1) * P],
                                    src, idn,
                                )
                    for pi, jp in enumerate((2 * half, 2 * half + 1)):
                        t_ = xtp.tile([P, 2 * CH], BF16, tag="xt", name=f"xt{jp}")
                        nc.vector.tensor_copy(out=t_, in_=pts[pi])
                        xt8[2 * jp] = t_[:, 0:CH]
                        xt8[2 * jp + 1] = t_[:, CH : 2 * CH]
                        if tb + 1 < NCH:
                            cast_block((tb + 1) * BPC + 2 * half + pi)
            xt_all[tb] = xt8

        vts_all = [None] * NCH

        def stage_B(tb):
            """q|k and v projections + PSUM->SBUF copies"""
            xt = xt_all[tb]
            pj_copy = nc.scalar.copy if tb < 2 else nc.vector.tensor_copy
            pqk = ppj.tile([P, CH], F32, tag="pj")
            for jc in range(NC):
                nc.tensor.matmul(pqk, lhsT=wqk_sb[:, jc, :], rhs=xt[jc],
                                 start=(jc == 0), stop=(jc == NC - 1))
            pj_copy(out=qT_c[tb], in_=pqk[0:H, :])
            pj_copy(out=kT_c[tb], in_=pqk[H : 2 * H, :])
            pv = ppj.tile([P, CH], F32, tag="pj")
            for jc in range(NC):
                nc.tensor.matmul(pv[0:H, :], lhsT=wv_sb[:, jc, :], rhs=xt[jc],
                                 start=(jc == 0), stop=(jc == NC - 1))
            vts = vtsp.tile([H, CH], BF16, tag="vt")
            pj_copy(out=vts, in_=pv[0:H, :])
            vts_all[tb] = vts

        def stage_vt(tb):
            """small transposes to v natural [s, 64] (vts copy has settled)"""
            vts = vts_all[tb]
            pvn = ptx.tile([P, 2 * CH], BF16, tag="tr")
            for tt in range(BPC):
                nc.tensor.transpose(pvn[:, tt * H : (tt + 1) * H],
                                    vts[:, tt * P : (tt + 1) * P],
                                    ident_b[0:H, 0:H])
            nc.vector.tensor_copy(out=v_c[tb][:, :, 0:H], in_=pvn[:, 0 : BPC * H])

        def stage_C(tb):
            """scores (2 s-blocks per [128,1024] f32 tile) + exp + mask + PV;
            diagonal s-blocks first so the chunk's closing exp->PV chain has no
            mask step and the irregular work happens early"""
            po = pout.tile([H + 1, CH], F32, tag="po")
            nsb = (tb + 1) * BPC
            sis = list(range(nsb))
            for g in range(nsb // 2):
                pair = sis[2 * g : 2 * g + 2]
                ps = psc.tile([P, 2 * CH], F32, tag="sc")
                w = wei.tile([P, 2 * CH], BF16, tag="w")
                los = []
                for m, si in enumerate(pair):
                    lo = max(0, (si - tb * BPC) * P)
                    los.append(lo)
                    nc.tensor.matmul(
                        ps[:, m * CH + lo : (m + 1) * CH],
                        lhsT=kT_c[si // BPC][:, (si % BPC) * P : (si % BPC + 1) * P],
                        rhs=qT_c[tb][:, lo:CH],
                        start=True, stop=True,
                    )
                # one exp over both blocks (the sub-lo slice of block m=1 is
                # garbage but never read; PV streams only [lo:]) — except the
                # kernel's very last pair, split to shorten the exp->PV tail
                last_pair = (tb == NCH - 1) and (g == nsb // 2 - 1)
                if not last_pair:
                    nc.scalar.activation(out=w[:, los[0] : 2 * CH],
                                         in_=ps[:, los[0] : 2 * CH],
                                         func=EXP, scale=SCALE)
                for m, si in enumerate(pair):
                    lo = los[m]
                    if last_pair:
                        nc.scalar.activation(out=w[:, m * CH + lo : (m + 1) * CH],
                                             in_=ps[:, m * CH + lo : (m + 1) * CH],
                                             func=EXP, scale=SCALE)
                    if si >= tb * BPC:  # diagonal block: zero below-diagonal (t < s)
                        nc.gpsimd.affine_select(
                            out=w[:, m * CH + lo : m * CH + lo + P],
                            in_=w[:, m * CH + lo : m * CH + lo + P],
                            compare_op=mybir.AluOpType.is_ge,
                            fill=0.0,
                            base=0,
                            # keep where (col - row) >= 0
                            pattern=[[1, P]],
                            channel_multiplier=-1,
                        )
                    nc.tensor.matmul(po[:, lo:CH], lhsT=v_c[si // BPC][:, si % BPC, :],
                                     rhs=w[:, m * CH + lo : (m + 1) * CH],
                                     start=(g == 0 and m == 0), stop=(g == nsb // 2 - 1 and m == 1))
            os_ = fin.tile([H + 1, CH], F32, tag="ot")
            if tb == NCH - 1:
                nc.scalar.copy(out=os_, in_=po)
            else:
                nc.vector.tensor_copy(out=os_, in_=po)
            nc.sync.dma_start(out=out_d[:, tb * CH : (tb + 1) * CH], in_=os_)

        # software-pipelined emission: chunk tb+1's transposes sit between
        # chunk tb's projection and score phases in the PE FIFO, covering the
        # projection-copy WAR stalls
        stage_A(0)
        stage_B(0)
        for tb in range(NCH):
            if tb + 1 < NCH:
                stage_A(tb + 1)
            stage_vt(tb)
            stage_C(tb)
            if tb + 1 < NCH:
                stage_B(tb + 1)
    return nc


_NC_CACHE = []


def _get_nc():
    if not _NC_CACHE:
        nc = build_nc()
        nc.finalize()  # bacc compile: register allocation, DCE
        _NC_CACHE.append(nc)
    return _NC_CACHE[0]


def make_in_maps(inputs):
    import ml_dtypes
    x = np.ascontiguousarray(
        np.asarray(inputs["x"], dtype=np.float32).astype(ml_dtypes.bfloat16))
    wq = np.asarray(inputs["Wq"], dtype=np.float32)
    wk = np.asarray(inputs["Wk"], dtype=np.float32)
    wv = np.asarray(inputs["Wv"], dtype=np.float32)
    # host-side repack into the on-chip stationary layout [p, jc, h]
    wqk = np.ascontiguousarray(np.concatenate(
        [wq.reshape(NC, P, H).transpose(1, 0, 2), wk.reshape(NC, P, H).transpose(1, 0, 2)],
        axis=2))
    wv2 = np.ascontiguousarray(wv.reshape(NC, P, H).transpose(1, 0, 2))
    return [{"xb16": np.ascontiguousarray(x[b]), "Wqk": wqk, "Wv2": wv2} for b in range(B)]


def kernel(**inputs):
    from concourse.bass_utils import run_bass_kernel_spmd

    nc = _get_nc()
    res = run_bass_kernel_spmd(nc, make_in_maps(inputs), core_ids=list(range(B)))
    return postprocess([res.results[b]["outT"] for b in range(B)])


def postprocess(outTs):
    outs = []
    for oT in outTs:
        outs.append((oT[0:H, :] / oT[H : H + 1, :]).T.astype(np.float32))
    return np.stack(outs, axis=0)


if __name__ == "__main__":
    import os
    os.makedirs("/tmp/neffdir3", exist_ok=True)
    from concourse.bass_utils import compile_bass_kernel

    nc = _get_nc()
    print("build OK, instructions:",
          sum(len(bb.instructions) for bb in nc.m.functions[0].blocks))
    print("COMPILED:", compile_bass_kernel(nc, "/tmp/neffdir3"))
